# revision 1
# baseline (speedup 1.0000x reference)
"""ColorLoss (3D color histogram + L1) Trainium2 kernel.

Strategy (data-parallel over batch, 8 cores):
  - Core i processes image i ([3,1024,1024]) plus 1/8 of the style image
    ([3,128,1024] row-slice).
  - Per pixel, channel bin indices r,g,b in [0,16) are computed exactly with
    ACT Relu-chains (clamp of 8x+8 to [0, 15.4]) + float->int16 convert
    (round-to-nearest) of t-0.5, i.e. floor.
  - flat bin = r + 16 g + 256 b = key1 + 64*key2 with key1 = r + 16*(g&3),
    key2 = (g>>2) + 4*b; both in [0,64).
  - 4096-bin joint histogram = 64x64 outer-product accumulation on the
    TensorEngine: PSUM[m,n] += sum_px E1[px,m] * E2[px,n], where E1/E2 are
    64-wide indicator encodings of key1/key2. Two pixel-blocks are packed
    per matmul (M=N=128, two 64-wide diagonal blocks; off-diagonal garbage
    is discarded), so one [128x128x128] matmul pair covers 256 pixels.
  - E1 (stationary side, strided APs are fine): plane-major, generated by
    DVE tensor_scalar(is_equal) at 4x bf16 rate; some planes are generated
    on ScalarE as Sign(key1 - j) instead (any encoding whose span contains
    the one-hots works; a 64x64 inverse un-mixes on the host).
  - E2 (moving side, must be contiguous per pixel): pixel-major, generated
    by a single DVE tensor_tensor(is_equal) with a broadcast AP against a
    static iota row.
  - Host: tiny 64x64 un-mix per core, assemble histograms, L1 loss.
"""
import sys

sys.path.insert(0, "/opt/trn_rl_repo")
import os
import numpy as np
from contextlib import ExitStack

import ml_dtypes  # noqa: F401

# ---------------- tunables ----------------
T = 288            # pixels per partition per chunk (must be even)
N_ACT_PLANES = 46  # side1 planes generated on ScalarE (Sign) instead of DVE
H, W = 1024, 1024
HW = H * W
IMG_PP = HW // 128          # pixels per partition for one image (8192)
STY_PP = 128 * W // 128     # pixels per partition for the style slice (1024)

_cache = {}


def _act_plane_set():
    """Spread the ACT planes; verify the mixed encoding matrix is invertible."""
    if N_ACT_PLANES == 0:
        return [], np.eye(64)
    step = max(1, 64 // N_ACT_PLANES)
    act = list(range(2, 64, step))[:N_ACT_PLANES]
    k = np.arange(64)
    M1 = np.eye(64)
    for j in act:
        M1[j] = np.sign(k - j)
    cond = np.linalg.cond(M1)
    assert cond < 1e8, f"bad plane split, cond={cond}"
    return act, np.linalg.inv(M1)


def _build():
    import concourse.bacc as bacc
    import concourse.mybir as mybir
    from concourse.tile import TileContext

    F32 = mybir.dt.float32
    BF16 = mybir.dt.bfloat16
    I16 = mybir.dt.int16
    Alu = mybir.AluOpType
    Act = mybir.ActivationFunctionType

    act_planes, M1inv = _act_plane_set()
    act_set = set(act_planes)

    nc = bacc.Bacc("TRN2")
    img_d = nc.dram_tensor("img", [3, H, W], F32, kind="ExternalInput")
    sty_d = nc.dram_tensor("sty", [3, 128, W], F32, kind="ExternalInput")
    o_d = nc.dram_tensor("out", [2, 128, 128], F32, kind="ExternalOutput")

    img_v = [img_d[c, :, :].rearrange("(p r) w -> p (r w)", p=128) for c in range(3)]
    sty_v = [sty_d[c, :, :] for c in range(3)]

    def chunks(total):
        off = 0
        out = []
        while off < total:
            tc_ = min(T, total - off)
            out.append((off, tc_))
            off += tc_
        return out

    img_chunks = chunks(IMG_PP)
    sty_chunks = chunks(STY_PP)

    with TileContext(nc) as tc:
        with ExitStack() as ctx:
            xpool = ctx.enter_context(tc.tile_pool(name="x", bufs=3))
            tpool = ctx.enter_context(tc.tile_pool(name="t", bufs=2))
            ipool = ctx.enter_context(tc.tile_pool(name="i", bufs=2))
            kpool = ctx.enter_context(tc.tile_pool(name="k", bufs=2))
            e1pool = ctx.enter_context(tc.tile_pool(name="e1", bufs=2))
            e2pool = ctx.enter_context(tc.tile_pool(name="e2", bufs=2))
            cpool = ctx.enter_context(tc.tile_pool(name="c", bufs=1))
            opool = ctx.enter_context(tc.tile_pool(name="o", bufs=1))
            pspool = ctx.enter_context(tc.tile_pool(name="ps", bufs=2, space="PSUM"))

            # constants
            iota = cpool.tile([128, 64], BF16, tag="iota")
            for j in range(64):
                nc.vector.memset(iota[:, j : j + 1], float(j))
            bcl1 = cpool.tile([128, 1], F32, tag="bcl1")
            nc.vector.memset(bcl1[:], 7.4)
            bcl2 = cpool.tile([128, 1], F32, tag="bcl2")
            nc.vector.memset(bcl2[:], 15.4)
            bias_j = cpool.tile([128, 64], F32, tag="biasj")
            for j in act_planes:
                nc.vector.memset(bias_j[:, j : j + 1], -float(j))

            ps_img = pspool.tile([128, 128], F32)
            ps_sty = pspool.tile([128, 128], F32)

            def do_chunk(views, off, tcw, ps, start, stop):
                th = tcw // 2
                xt = xpool.tile([128, 3, T], F32, tag="xt")
                for c in range(3):
                    nc.sync.dma_start(xt[:, c, :tcw], views[c][:, off : off + tcw])
                ut = tpool.tile([128, 3, T], F32, tag="ut")
                tt = tpool.tile([128, 3, T], F32, tag="tt")
                for c in range(3):
                    nc.scalar.activation(ut[:, c, :tcw], xt[:, c, :tcw], Act.Relu,
                                         bias=bcl1[:], scale=-8.0)
                    nc.scalar.activation(tt[:, c, :tcw], ut[:, c, :tcw], Act.Relu,
                                         bias=bcl2[:], scale=-1.0)
                ii = ipool.tile([128, 4, T], I16, tag="ii")
                # floor via round(t - 0.5): rows r,g,b idx + gh
                for c in range(3):
                    nc.vector.tensor_scalar(ii[:, c, :tcw], tt[:, c, :tcw], 0.5, None,
                                            Alu.subtract)
                nc.vector.tensor_scalar(ii[:, 3, :tcw], tt[:, 1, :tcw], 0.25, 0.5,
                                        Alu.mult, Alu.subtract)
                kk = kpool.tile([128, 6, T], BF16, tag="kk")
                # rows: 0=r 1=g 2=b 3=gh 4=key1(after gl) 5=key2
                for c in range(4):
                    nc.vector.tensor_copy(kk[:, c, :tcw], ii[:, c, :tcw])
                # gl = g - 4*gh  (reuse row 1)
                nc.vector.scalar_tensor_tensor(kk[:, 1, :tcw], kk[:, 3, :tcw], -4.0,
                                               kk[:, 1, :tcw], Alu.mult, Alu.add)
                # key1 = 16*gl + r ; key2 = 4*b + gh
                nc.vector.scalar_tensor_tensor(kk[:, 4, :tcw], kk[:, 1, :tcw], 16.0,
                                               kk[:, 0, :tcw], Alu.mult, Alu.add)
                nc.vector.scalar_tensor_tensor(kk[:, 5, :tcw], kk[:, 2, :tcw], 4.0,
                                               kk[:, 3, :tcw], Alu.mult, Alu.add)
                key1 = kk[:, 4, :tcw]
                key2 = kk[:, 5, :tcw]

                # side1: plane-major [128, 64, 2, T//2]
                e1 = e1pool.tile([128, 64, 2, T // 2], BF16, tag="e1")
                k1v = key1.rearrange("p (b t) -> p b t", b=2)
                for j in range(64):
                    dst = e1[:, j, :, :th]
                    if j in act_set:
                        nc.scalar.activation(dst, k1v, Act.Sign,
                                             bias=bias_j[:, j : j + 1], scale=1.0)
                    else:
                        nc.vector.tensor_scalar(dst, k1v, float(j), None, Alu.is_equal)

                # side2: pixel-major [128, T//2, 2, 64] in one broadcast TT
                e2 = e2pool.tile([128, T // 2, 2, 64], BF16, tag="e2")
                k2v = key2.rearrange("p (b t) -> p t b", b=2)  # [128, th, 2]
                k2b = k2v.unsqueeze(3).broadcast_to([128, th, 2, 64])
                i2b = iota[:, :].unsqueeze(1).unsqueeze(2).broadcast_to([128, th, 2, 64])
                nc.vector.tensor_tensor(e2[:, :th, :, :], k2b, i2b, Alu.is_equal)

                for tau in range(th):
                    nc.tensor.matmul(
                        ps[:],
                        e1[:, :, :, tau],
                        e2[:, tau, :, :],
                        start=(start and tau == 0),
                        stop=(stop and tau == th - 1),
                    )

            n_img = len(img_chunks)
            for ci, (off, tcw) in enumerate(img_chunks):
                do_chunk(img_v, off, tcw, ps_img, ci == 0, ci == n_img - 1)
            n_sty = len(sty_chunks)
            for ci, (off, tcw) in enumerate(sty_chunks):
                do_chunk(sty_v, off, tcw, ps_sty, ci == 0, ci == n_sty - 1)

            ostage = opool.tile([128, 2, 128], F32)
            nc.vector.tensor_copy(ostage[:, 0, :], ps_img[:])
            nc.vector.tensor_copy(ostage[:, 1, :], ps_sty[:])
            nc.sync.dma_start(o_d[0, :, :], ostage[:, 0, :])
            nc.sync.dma_start(o_d[1, :, :], ostage[:, 1, :])

    nc.finalize()
    return nc, M1inv


def _get_built():
    if "nc" not in _cache:
        nc, M1inv = _build()
        _cache["nc"] = nc
        _cache["M1inv"] = M1inv
    return _cache["nc"], _cache["M1inv"]


def _unmix(raw, M1inv):
    """raw [2,128,128] f32 -> (hist_img[4096], hist_sty[4096]) exact counts."""
    out = []
    for s in range(2):
        r = raw[s].astype(np.float64)
        mixed = r[0::2, 0:64] + r[1::2, 64:128]   # [64 j1, 64 j2]
        Hm = M1inv @ mixed
        out.append(np.rint(Hm))
    return out


def kernel(input, style_image, n_bins):
    assert int(n_bins) == 16
    from concourse import bass_utils

    nc, M1inv = _get_built()
    input = np.ascontiguousarray(np.asarray(input, dtype=np.float32))
    style = np.ascontiguousarray(np.asarray(style_image, dtype=np.float32))
    B = input.shape[0]
    assert B == 8 and input.shape == (8, 3, H, W)
    in_maps = [
        {
            "img": input[i],
            "sty": np.ascontiguousarray(style[0, :, 128 * i : 128 * (i + 1), :]),
        }
        for i in range(8)
    ]
    res = bass_utils.run_bass_kernel_spmd(nc, in_maps, core_ids=list(range(8)),
                                          **_cache.get("run_kwargs", {}))
    _cache["last_results"] = res
    hists = np.zeros((B, 4096), np.float64)
    sty_hist = np.zeros(4096, np.float64)
    for i in range(8):
        hi, hs = _unmix(res.results[i]["out"], M1inv)
        # flat = key1 + 64*key2 -> hist_flat[f] = H[j1=f%64, j2=f//64]
        hists[i] = hi.T.reshape(4096)
        sty_hist += hs.T.reshape(4096)
    cols = (hists / HW).astype(np.float32)
    target = (sty_hist / HW).astype(np.float32)
    loss = np.mean(np.abs(cols - target[None, :]).astype(np.float32))
    return np.float32(loss)



# revision 5
# speedup vs baseline: 1.0797x; 1.0797x over previous
"""ColorLoss (3D color histogram + L1) Trainium2 kernel — v2.

Strategy (data-parallel over batch, 8 cores):
  - Core i processes image i ([3,1024,1024]) plus 1/8 of the style image
    ([3,128,1024] row-slice).  4096 bins = 64x64 via key1 = 4*h1+rl,
    key2 = 4*b+gh with h1 = rh + 4*gl, rh=r>>2, rl=r&3, gl=g&3, gh=g>>2.
  - Encodings are EXACT fp8(e4m3) one-hots packed 4-planes-per-int32:
    word_e = (h==e) * (56 << 8*lo), byte pattern 0x38 = fp8 1.0 at byte lo.
    16 scalar_tensor_tensor ops per side per chunk write all 64 planes.
  - Per-pixel floors (r, g, b, r>>2, g>>2) computed on the Scalar engine as
    activation Copy with bias=-0.5 and int output (round -> floor).
  - Integer helper chains (rl, gl, h1, 56*256^lo) run on GPSIMD tensor_tensor
    + a few DVE tensor_scalar ops, keeping DVE mostly on the 32 stt plane ops.
  - Matmul: fp8 DoubleRow, 512 px/instr: out[m=(c,key1), n=(c',key2)] over
    [K=128 partitions x 2 slabs]; 2 c-blocks diagonal; off-diagonal junk
    discarded on host.  PSUM [128,128] f32, counts exact.
"""
import sys

sys.path.insert(0, "/opt/trn_rl_repo")
import os
import numpy as np
from contextlib import ExitStack

import ml_dtypes  # noqa: F401

# ---------------- tunables ----------------
T = 512             # pixels per partition per chunk (multiple of 4)
H, W = 1024, 1024
HW = H * W
IMG_PP = HW // 128          # 8192 pixels/partition for one image
STY_PP = 128 * W // 128     # 1024 pixels/partition for the style slice

_cache = {}


def _build():
    import concourse.bacc as bacc
    import concourse.mybir as mybir
    from concourse.tile import TileContext

    F32 = mybir.dt.float32
    I32 = mybir.dt.int32
    I16 = mybir.dt.int16
    FP8 = mybir.dt.float8e4
    Alu = mybir.AluOpType
    Act = mybir.ActivationFunctionType
    DR = mybir.MatmulPerfMode.DoubleRow

    Q = T // 4

    nc = bacc.Bacc("TRN2")
    img_d = nc.dram_tensor("img", [3, H, W], F32, kind="ExternalInput")
    sty_d = nc.dram_tensor("sty", [3, 128, W], F32, kind="ExternalInput")
    o_d = nc.dram_tensor("out", [2, 128, 128], F32, kind="ExternalOutput")

    img_v = [img_d[c, :, :].rearrange("(p r) w -> p (r w)", p=128) for c in range(3)]
    sty_v = [sty_d[c, :, :] for c in range(3)]

    with TileContext(nc) as tc:
        with ExitStack() as ctx:
            xpool = ctx.enter_context(tc.tile_pool(name="x", bufs=2))
            fpool = ctx.enter_context(tc.tile_pool(name="f", bufs=2))
            bpool = ctx.enter_context(tc.tile_pool(name="b", bufs=2))
            kpool = ctx.enter_context(tc.tile_pool(name="k", bufs=2))
            jpool = ctx.enter_context(tc.tile_pool(name="j", bufs=1))
            epool = ctx.enter_context(tc.tile_pool(name="e", bufs=2))
            cpool = ctx.enter_context(tc.tile_pool(name="c", bufs=1))
            opool = ctx.enter_context(tc.tile_pool(name="o", bufs=1))
            pspool = ctx.enter_context(tc.tile_pool(name="ps", bufs=2, space="PSUM"))

            # constants
            bcl1 = cpool.tile([128, 1], F32, tag="bcl1")
            nc.vector.memset(bcl1[:], 7.4)
            bcl2 = cpool.tile([128, 1], F32, tag="bcl2")
            nc.vector.memset(bcl2[:], 15.4)
            c255 = cpool.tile([128, 1], I16, tag="c255")
            nc.vector.memset(c255[:], 255)
            c65535 = cpool.tile([128, 1], I32, tag="c65535")
            nc.vector.memset(c65535[:], 65535)

            ps_img = pspool.tile([128, 128], F32)
            ps_sty = pspool.tile([128, 128], F32)

            def do_chunk(views, off, ps, start, stop):
                xt = xpool.tile([128, 3, T], F32, tag="xt")
                for c in range(3):
                    nc.sync.dma_start(xt[:, c, :], views[c][:, off : off + T])

                # ACT: t = clamp(8x+8, 0, 15.4)  (u = Relu(-8x+7.4), in-place)
                tt = fpool.tile([128, 3, T], F32, tag="tt")
                for c in range(3):
                    nc.scalar.activation(tt[:, c, :], xt[:, c, :], Act.Relu,
                                         bias=bcl1[:], scale=-8.0)
                    nc.scalar.activation(tt[:, c, :], tt[:, c, :], Act.Relu,
                                         bias=bcl2[:], scale=-1.0)

                # ACT floors: r,g,rh,gh,h2=b (all i32)
                bins = bpool.tile([128, 2, T], I32, tag="bins")    # r, g
                bins4 = bpool.tile([128, 2, T], I32, tag="bins4")  # rh, gh
                h2 = kpool.tile([128, T], I32, tag="h2")           # b bin
                nc.scalar.activation(bins[:, 0, :], tt[:, 0, :], Act.Copy,
                                     bias=-0.5, scale=1.0)
                nc.scalar.activation(bins[:, 1, :], tt[:, 1, :], Act.Copy,
                                     bias=-0.5, scale=1.0)
                nc.scalar.activation(bins4[:, 0, :], tt[:, 0, :], Act.Copy,
                                     bias=-0.5, scale=0.25)
                nc.scalar.activation(bins4[:, 1, :], tt[:, 1, :], Act.Copy,
                                     bias=-0.5, scale=0.25)
                nc.scalar.activation(h2[:], tt[:, 2, :], Act.Copy,
                                     bias=-0.5, scale=1.0)

                r = bins[:, 0, :]
                g = bins[:, 1, :]
                rh = bins4[:, 0, :]
                gh = bins4[:, 1, :]

                # Pool (GPSIMD) integer chains: rl = r-4rh, gl = g-4gh,
                # h1 = rh+4gl  (all i32, via adds/subtract)
                pi = kpool.tile([128, 4, T], I32, tag="pi")
                # pi rows: 0=scratch 1=rl 2=gl 3=h1
                nc.gpsimd.tensor_tensor(pi[:, 0, :], rh, rh, Alu.add)          # 2rh
                nc.gpsimd.tensor_tensor(pi[:, 0, :], pi[:, 0, :], pi[:, 0, :],
                                        Alu.add)                               # 4rh
                nc.gpsimd.tensor_tensor(pi[:, 1, :], r, pi[:, 0, :],
                                        Alu.subtract)                          # rl
                nc.gpsimd.tensor_tensor(pi[:, 0, :], gh, gh, Alu.add)          # 2gh
                nc.gpsimd.tensor_tensor(pi[:, 0, :], pi[:, 0, :], pi[:, 0, :],
                                        Alu.add)                               # 4gh
                nc.gpsimd.tensor_tensor(pi[:, 2, :], g, pi[:, 0, :],
                                        Alu.subtract)                          # gl
                nc.gpsimd.tensor_tensor(pi[:, 0, :], pi[:, 2, :], pi[:, 2, :],
                                        Alu.add)                               # 2gl
                nc.gpsimd.tensor_tensor(pi[:, 0, :], pi[:, 0, :], pi[:, 0, :],
                                        Alu.add)                               # 4gl
                nc.gpsimd.tensor_tensor(pi[:, 3, :], rh, pi[:, 0, :], Alu.add) # h1

                rl = pi[:, 1, :]
                gl = pi[:, 2, :]
                h1 = pi[:, 3, :]

                # DVE: t1/t2 multiplier chains (i32; bitwise is DVE-32bit-only)
                lo = jpool.tile([128, 2, T], I32, tag="lo")  # l0,l1 (reused)
                m0 = jpool.tile([128, 2, T], I16, tag="m0")
                m1 = jpool.tile([128, 2, T], I32, tag="m1")
                nc.vector.tensor_scalar(lo[:, 0, :], rl, 1, None, Alu.bitwise_and)
                nc.vector.tensor_scalar(lo[:, 1, :], rl, 1, None,
                                        Alu.logical_shift_right)
                nc.vector.tensor_scalar(m0[:, 0, :], lo[:, 0, :], 255, 1,
                                        Alu.mult, Alu.add)
                nc.vector.tensor_scalar(m1[:, 0, :], lo[:, 1, :], 65535, 1,
                                        Alu.mult, Alu.add)
                nc.vector.tensor_scalar(lo[:, 0, :], gh, 1, None, Alu.bitwise_and)
                nc.vector.tensor_scalar(lo[:, 1, :], gh, 1, None,
                                        Alu.logical_shift_right)
                nc.vector.tensor_scalar(m0[:, 1, :], lo[:, 0, :], 255, 1,
                                        Alu.mult, Alu.add)
                nc.vector.tensor_scalar(m1[:, 1, :], lo[:, 1, :], 65535, 1,
                                        Alu.mult, Alu.add)
                # t = 56 * m0 * m1 (i32) -> fp8 1.0 byte at position lo
                t1 = kpool.tile([128, T], I32, tag="t1")
                t2 = kpool.tile([128, T], I32, tag="t2")
                nc.vector.scalar_tensor_tensor(t1[:], m0[:, 0, :], 56, m1[:, 0, :],
                                               Alu.mult, Alu.mult)
                nc.vector.scalar_tensor_tensor(t2[:], m0[:, 1, :], 56, m1[:, 1, :],
                                               Alu.mult, Alu.mult)

                # DVE: packed planes, pixel t=(q,s,c)
                h1v = h1[:].rearrange("p (q s c) -> p q s c", s=2, c=2)
                t1v = t1[:].rearrange("p (q s c) -> p q s c", s=2, c=2)
                h2v = h2[:].rearrange("p (q s c) -> p q s c", s=2, c=2)
                t2v = t2[:].rearrange("p (q s c) -> p q s c", s=2, c=2)
                e1p = epool.tile([128, Q, 2, 2, 16], I32, tag="e1p")
                e2p = epool.tile([128, Q, 2, 2, 16], I32, tag="e2p")
                for e in range(16):
                    nc.vector.scalar_tensor_tensor(e1p[:, :, :, :, e], h1v, e, t1v,
                                                   Alu.is_equal, Alu.mult)
                    nc.vector.scalar_tensor_tensor(e2p[:, :, :, :, e], h2v, e, t2v,
                                                   Alu.is_equal, Alu.mult)

                e1f = e1p[:, :, :, :, :].bitcast(FP8).rearrange(
                    "p q s c eb -> p q s (c eb)")
                e2f = e2p[:, :, :, :, :].bitcast(FP8).rearrange(
                    "p q s c eb -> p q s (c eb)")
                for q in range(Q):
                    nc.tensor.matmul(ps[:], e1f[:, q, :, :], e2f[:, q, :, :],
                                     start=(start and q == 0),
                                     stop=(stop and q == Q - 1),
                                     perf_mode=DR)

            n_img = IMG_PP // T
            for ci in range(n_img):
                do_chunk(img_v, ci * T, ps_img, ci == 0, ci == n_img - 1)
            n_sty = STY_PP // T
            for ci in range(n_sty):
                do_chunk(sty_v, ci * T, ps_sty, ci == 0, ci == n_sty - 1)

            ostage = opool.tile([128, 2, 128], F32)
            nc.vector.tensor_copy(ostage[:, 0, :], ps_img[:])
            nc.vector.tensor_copy(ostage[:, 1, :], ps_sty[:])
            nc.sync.dma_start(o_d[0, :, :], ostage[:, 0, :])
            nc.sync.dma_start(o_d[1, :, :], ostage[:, 1, :])

    nc.finalize()
    return nc


def _get_built():
    if "nc" not in _cache:
        _cache["nc"] = _build()
    return _cache["nc"]


def _perm():
    """flat[key1, key2] -> flat bin index (r + 16g + 256b)."""
    if "perm" in _cache:
        return _cache["perm"]
    k1 = np.arange(64)
    k2 = np.arange(64)
    rl = k1 & 3
    h1 = k1 >> 2
    rh = h1 & 3
    gl = h1 >> 2
    r = 4 * rh + rl            # [64]
    gh = k2 & 3
    b = k2 >> 2
    flat = (r[:, None] + 16 * (gl[:, None] + 4 * gh[None, :])
            + 256 * b[None, :])  # [64,64]
    _cache["perm"] = flat
    return flat


def _decode(raw):
    """raw [2,128,128] f32 -> (hist_img[4096], hist_sty[4096]) exact counts."""
    flat = _perm()
    out = []
    for s_ in range(2):
        m = raw[s_]
        counts64 = m[0:64, 0:64] + m[64:128, 64:128]   # [key1, key2]
        h = np.zeros(4096)
        np.add.at(h, flat.reshape(-1), counts64.reshape(-1))
        out.append(h)
    return out


def kernel(input, style_image, n_bins):
    assert int(n_bins) == 16
    from concourse import bass_utils

    nc = _get_built()
    input = np.ascontiguousarray(np.asarray(input, dtype=np.float32))
    style = np.ascontiguousarray(np.asarray(style_image, dtype=np.float32))
    B = input.shape[0]
    assert B == 8 and input.shape == (8, 3, H, W)
    in_maps = [
        {
            "img": input[i],
            "sty": np.ascontiguousarray(style[0, :, 128 * i : 128 * (i + 1), :]),
        }
        for i in range(8)
    ]
    res = bass_utils.run_bass_kernel_spmd(nc, in_maps, core_ids=list(range(8)),
                                          **_cache.get("run_kwargs", {}))
    _cache["last_results"] = res
    hists = np.zeros((B, 4096))
    sty_hist = np.zeros(4096)
    for i in range(8):
        hi, hs = _decode(res.results[i]["out"])
        hists[i] = hi
        sty_hist += hs
    cols = (hists / HW).astype(np.float32)
    target = (sty_hist / HW).astype(np.float32)
    loss = np.mean(np.abs(cols - target[None, :]).astype(np.float32))
    return np.float32(loss)


# revision 7
# speedup vs baseline: 1.2768x; 1.1826x over previous
"""ColorLoss (3D color histogram + L1) Trainium2 kernel — v2.

Strategy (data-parallel over batch, 8 cores):
  - Core i processes image i ([3,1024,1024]) plus 1/8 of the style image
    ([3,128,1024] row-slice).  4096 bins = 64x64 via key1 = 4*h1+rl,
    key2 = 4*b+gh with h1 = rh + 4*gl, rh=r>>2, rl=r&3, gl=g&3, gh=g>>2.
  - Encodings are EXACT fp8(e4m3) one-hots packed 4-planes-per-int32:
    word_e = (h==e) * (56 << 8*lo), byte pattern 0x38 = fp8 1.0 at byte lo.
    16 scalar_tensor_tensor ops per side per chunk write all 64 planes.
  - Per-pixel floors (r, g, b, r>>2, g>>2) computed on the Scalar engine as
    activation Copy with bias=-0.5 and int output (round -> floor).
  - Integer helper chains (rl, gl, h1, 56*256^lo) run on GPSIMD tensor_tensor
    + a few DVE tensor_scalar ops, keeping DVE mostly on the 32 stt plane ops.
  - Matmul: fp8 DoubleRow, 512 px/instr: out[m=(c,key1), n=(c',key2)] over
    [K=128 partitions x 2 slabs]; 2 c-blocks diagonal; off-diagonal junk
    discarded on host.  PSUM [128,128] f32, counts exact.
"""
import sys

sys.path.insert(0, "/opt/trn_rl_repo")
import os
import numpy as np
from contextlib import ExitStack

import ml_dtypes  # noqa: F401

# ---------------- tunables ----------------
T = 512             # pixels per partition per chunk (multiple of 4)
H, W = 1024, 1024
HW = H * W
IMG_PP = HW // 128          # 8192 pixels/partition for one image
STY_PP = 128 * W // 128     # 1024 pixels/partition for the style slice

_cache = {}


def _build():
    import concourse.bacc as bacc
    import concourse.mybir as mybir
    from concourse.tile import TileContext

    F32 = mybir.dt.float32
    I32 = mybir.dt.int32
    I16 = mybir.dt.int16
    FP8 = mybir.dt.float8e4
    Alu = mybir.AluOpType
    Act = mybir.ActivationFunctionType
    DR = mybir.MatmulPerfMode.DoubleRow

    Q = T // 4

    nc = bacc.Bacc("TRN2")
    img_d = nc.dram_tensor("img", [3, H, W], F32, kind="ExternalInput")
    sty_d = nc.dram_tensor("sty", [3, 128, W], F32, kind="ExternalInput")
    o_d = nc.dram_tensor("out", [2, 128, 128], F32, kind="ExternalOutput")

    img_v = [img_d[c, :, :].rearrange("(p r) w -> p (r w)", p=128) for c in range(3)]
    sty_v = [sty_d[c, :, :] for c in range(3)]

    with TileContext(nc) as tc:
        with ExitStack() as ctx:
            xpool = ctx.enter_context(tc.tile_pool(name="x", bufs=2))
            fpool = ctx.enter_context(tc.tile_pool(name="f", bufs=2))
            bpool = ctx.enter_context(tc.tile_pool(name="b", bufs=2))
            kpool = ctx.enter_context(tc.tile_pool(name="k", bufs=2))
            jpool = ctx.enter_context(tc.tile_pool(name="j", bufs=1))
            epool = ctx.enter_context(tc.tile_pool(name="e", bufs=2))
            cpool = ctx.enter_context(tc.tile_pool(name="c", bufs=1))
            opool = ctx.enter_context(tc.tile_pool(name="o", bufs=1))
            pspool = ctx.enter_context(tc.tile_pool(name="ps", bufs=2, space="PSUM"))

            # constants
            bcl1 = cpool.tile([128, 1], F32, tag="bcl1")
            nc.vector.memset(bcl1[:], 7.4)
            bcl2 = cpool.tile([128, 1], F32, tag="bcl2")
            nc.vector.memset(bcl2[:], 15.4)
            c56 = cpool.tile([128, 1], I32, tag="c56")
            nc.vector.memset(c56[:], 56)

            ps_img = pspool.tile([128, 128], F32)
            ps_sty = pspool.tile([128, 128], F32)

            def do_chunk(views, off, ps, start, stop):
                xt = xpool.tile([128, 3, T], F32, tag="xt")
                for c in range(3):
                    nc.sync.dma_start(xt[:, c, :], views[c][:, off : off + T])

                # ACT: t = clamp(8x+8, 0, 15.4)  (u = Relu(-8x+7.4), in-place)
                tt = fpool.tile([128, 3, T], F32, tag="tt")
                for c in range(3):
                    nc.scalar.activation(tt[:, c, :], xt[:, c, :], Act.Relu,
                                         bias=bcl1[:], scale=-8.0)
                    nc.scalar.activation(tt[:, c, :], tt[:, c, :], Act.Relu,
                                         bias=bcl2[:], scale=-1.0)

                # ACT floors: r,g,rh,gh,h2=b (all i32)
                bins = bpool.tile([128, 2, T], I32, tag="bins")    # r, g
                bins4 = bpool.tile([128, 2, T], I32, tag="bins4")  # rh, gh
                h2 = kpool.tile([128, T], I32, tag="h2")           # b bin
                nc.scalar.activation(bins[:, 0, :], tt[:, 0, :], Act.Copy,
                                     bias=-0.5, scale=1.0)
                nc.scalar.activation(bins[:, 1, :], tt[:, 1, :], Act.Copy,
                                     bias=-0.5, scale=1.0)
                nc.scalar.activation(bins4[:, 0, :], tt[:, 0, :], Act.Copy,
                                     bias=-0.5, scale=0.25)
                nc.scalar.activation(bins4[:, 1, :], tt[:, 1, :], Act.Copy,
                                     bias=-0.5, scale=0.25)
                nc.scalar.activation(h2[:], tt[:, 2, :], Act.Copy,
                                     bias=-0.5, scale=1.0)

                r = bins[:, 0, :]
                g = bins[:, 1, :]
                rh = bins4[:, 0, :]
                gh = bins4[:, 1, :]

                # Pool (GPSIMD) integer chains: rl = r-4rh, gl = g-4gh,
                # h1 = rh+4gl  (all i32, via adds/subtract)
                pi = kpool.tile([128, 4, T], I32, tag="pi")
                # pi rows: 0=scratch 1=rl 2=gl 3=h1
                nc.gpsimd.tensor_tensor(pi[:, 0, :], rh, rh, Alu.add)          # 2rh
                nc.gpsimd.tensor_tensor(pi[:, 0, :], pi[:, 0, :], pi[:, 0, :],
                                        Alu.add)                               # 4rh
                nc.gpsimd.tensor_tensor(pi[:, 1, :], r, pi[:, 0, :],
                                        Alu.subtract)                          # rl
                nc.gpsimd.tensor_tensor(pi[:, 0, :], gh, gh, Alu.add)          # 2gh
                nc.gpsimd.tensor_tensor(pi[:, 0, :], pi[:, 0, :], pi[:, 0, :],
                                        Alu.add)                               # 4gh
                nc.gpsimd.tensor_tensor(pi[:, 2, :], g, pi[:, 0, :],
                                        Alu.subtract)                          # gl
                nc.gpsimd.tensor_tensor(pi[:, 0, :], pi[:, 2, :], pi[:, 2, :],
                                        Alu.add)                               # 2gl
                nc.gpsimd.tensor_tensor(pi[:, 0, :], pi[:, 0, :], pi[:, 0, :],
                                        Alu.add)                               # 4gl
                nc.gpsimd.tensor_tensor(pi[:, 3, :], rh, pi[:, 0, :], Alu.add) # h1

                rl = pi[:, 1, :]
                gl = pi[:, 2, :]
                h1 = pi[:, 3, :]

                # DVE: t = 56 << (8*lo) via mult + TT shift (broadcast const)
                s8 = jpool.tile([128, 2, T], I32, tag="s8")
                t1 = kpool.tile([128, T], I32, tag="t1")
                t2 = kpool.tile([128, T], I32, tag="t2")
                c56b = c56[:].broadcast_to([128, T])
                nc.vector.tensor_scalar(s8[:, 0, :], rl, 8, None, Alu.mult)
                nc.vector.tensor_tensor(t1[:], c56b, s8[:, 0, :],
                                        Alu.logical_shift_left)
                nc.vector.tensor_scalar(s8[:, 1, :], gh, 8, None, Alu.mult)
                nc.vector.tensor_tensor(t2[:], c56b, s8[:, 1, :],
                                        Alu.logical_shift_left)

                # DVE: packed planes, pixel t=(q,s,c)
                h1v = h1.rearrange("p (q s c) -> p q s c", s=2, c=2)
                t1v = t1[:].rearrange("p (q s c) -> p q s c", s=2, c=2)
                e1p = epool.tile([128, Q, 2, 2, 16], I32, tag="e1p")
                e2c = epool.tile([128, 16, T], I32, tag="e2c")
                for e in range(16):
                    nc.vector.scalar_tensor_tensor(e1p[:, :, :, :, e], h1v, e, t1v,
                                                   Alu.is_equal, Alu.mult)
                    nc.vector.scalar_tensor_tensor(e2c[:, e, :], h2[:], e, t2[:],
                                                   Alu.is_equal, Alu.mult)

                e1f = e1p[:, :, :, :, :].bitcast(FP8).rearrange(
                    "p q s c eb -> p q s (c eb)")
                e2f = e2c[:, :, :].bitcast(FP8).rearrange(
                    "p e (q s cb) -> p q s e cb", s=2, cb=8)
                for q in range(Q):
                    nc.tensor.matmul(ps[:], e1f[:, q, :, :], e2f[:, q, :, :, :],
                                     start=(start and q == 0),
                                     stop=(stop and q == Q - 1),
                                     perf_mode=DR)

            n_img = IMG_PP // T
            for ci in range(n_img):
                do_chunk(img_v, ci * T, ps_img, ci == 0, ci == n_img - 1)
            n_sty = STY_PP // T
            for ci in range(n_sty):
                do_chunk(sty_v, ci * T, ps_sty, ci == 0, ci == n_sty - 1)

            ostage = opool.tile([128, 2, 128], F32)
            nc.vector.tensor_copy(ostage[:, 0, :], ps_img[:])
            nc.vector.tensor_copy(ostage[:, 1, :], ps_sty[:])
            nc.sync.dma_start(o_d[0, :, :], ostage[:, 0, :])
            nc.sync.dma_start(o_d[1, :, :], ostage[:, 1, :])

    nc.finalize()
    return nc


def _get_built():
    if "nc" not in _cache:
        _cache["nc"] = _build()
    return _cache["nc"]


def _perm():
    """flat[key1, key2] -> flat bin index (r + 16g + 256b)."""
    if "perm" in _cache:
        return _cache["perm"]
    k1 = np.arange(64)
    k2 = np.arange(64)
    rl = k1 & 3
    h1 = k1 >> 2
    rh = h1 & 3
    gl = h1 >> 2
    r = 4 * rh + rl            # [64]
    gh = k2 & 3
    b = k2 >> 2
    flat = (r[:, None] + 16 * (gl[:, None] + 4 * gh[None, :])
            + 256 * b[None, :])  # [64,64]
    _cache["perm"] = flat
    return flat


def _col_idx():
    if "colidx" not in _cache:
        k2 = np.arange(64)
        _cache["colidx"] = [(k2 // 4) * 8 + c * 4 + (k2 % 4) for c in range(2)]
    return _cache["colidx"]


def _decode(raw):
    """raw [2,128,128] f32 -> (hist_img[4096], hist_sty[4096]) exact counts."""
    flat = _perm()
    n0, n1 = _col_idx()
    out = []
    for s_ in range(2):
        m = raw[s_]
        counts64 = m[0:64, :][:, n0] + m[64:128, :][:, n1]   # [key1, key2]
        h = np.zeros(4096)
        np.add.at(h, flat.reshape(-1), counts64.reshape(-1))
        out.append(h)
    return out


def kernel(input, style_image, n_bins):
    assert int(n_bins) == 16
    from concourse import bass_utils

    nc = _get_built()
    input = np.ascontiguousarray(np.asarray(input, dtype=np.float32))
    style = np.ascontiguousarray(np.asarray(style_image, dtype=np.float32))
    B = input.shape[0]
    assert B == 8 and input.shape == (8, 3, H, W)
    in_maps = [
        {
            "img": input[i],
            "sty": np.ascontiguousarray(style[0, :, 128 * i : 128 * (i + 1), :]),
        }
        for i in range(8)
    ]
    res = bass_utils.run_bass_kernel_spmd(nc, in_maps, core_ids=list(range(8)),
                                          **_cache.get("run_kwargs", {}))
    _cache["last_results"] = res
    hists = np.zeros((B, 4096))
    sty_hist = np.zeros(4096)
    for i in range(8):
        hi, hs = _decode(res.results[i]["out"])
        hists[i] = hi
        sty_hist += hs
    cols = (hists / HW).astype(np.float32)
    target = (sty_hist / HW).astype(np.float32)
    loss = np.mean(np.abs(cols - target[None, :]).astype(np.float32))
    return np.float32(loss)
